# revision 36
# baseline (speedup 1.0000x reference)
"""Trainium2 Bass kernel for a causal attention block (B=2, T=2048, E=2048,
16 heads, head_dim=128, interleaved RoPE).

Sharding: data-parallel over batch (2) x tensor-parallel over heads (4 per
core) = 8 NeuronCores. Each core computes QKV projection for its 4 heads,
RoPE, causal SDPA, and a partial output projection (row-sharded W_out); the
host sums the 4 TP partials per batch element.

Per-core dataflow (fp16 matmul inputs, fp32 PSUM accumulation):
  phase 1: QT/KT computed transposed ([head_dim, T], rows de-interleaved so
           RoPE is a top/bottom-half rotation applied at PSUM eviction) and
           V natural [T, head_dim]; all stay resident in SBUF.
  phase 2: per head: scoresT[Tk,Tq] = KT_tile.T @ QT, exp (scale folded in),
           causal mask on the diagonal blocks, denominator accumulated on
           DVE + ones-matmul + fast reciprocal + gpsimd partition broadcast;
           exp-scores feed Y.T[D,Tq] = V.T @ A.T directly, normalized on
           eviction.
  phase 3: out[T, E] partial = Y.T (as lhsT) against W_out local rows.
"""

import sys

sys.path.insert(0, "/opt/trn_rl_repo")

import numpy as np

import bass_rust
import concourse.bacc as bacc
import concourse.mybir as mybir
from concourse.alu_op_type import AluOpType
from concourse import tile
from concourse import bass_utils

B, T, E = 2, 2048, 2048
N_HEAD = 16
D = E // N_HEAD            # 128
THETA = 10000.0
N_CORES = 8
TP = 4                     # tensor-parallel degree (heads)
HPC = N_HEAD // TP         # heads per core = 4
FL = HPC * D               # local head width = 512
EC = E // 128              # 16 contraction chunks
TQ = 512                   # query tile (free dim)
NTQ = T // TQ              # 4
NTK = T // 128             # 16

F32 = mybir.dt.float32
F32R = mybir.dt.float32r
F16 = mybir.dt.float16
EXP = mybir.ActivationFunctionType.Exp
SCALE = 1.0 / np.sqrt(D)

_compiled = None
_last_in_maps = None


def _build():
    nc = bacc.Bacc("TRN2", target_bir_lowering=False)

    xT = nc.dram_tensor("xT", (E, T), F16, kind="ExternalInput")
    wqk = nc.dram_tensor("wqk", (EC, 128, 2 * FL), F16, kind="ExternalInput")
    wv = nc.dram_tensor("wv", (EC, 128, FL), F16, kind="ExternalInput")
    wout = nc.dram_tensor("wout", (HPC, 128, E), F16, kind="ExternalInput")
    csx = nc.dram_tensor("csx", (128, T), F16, kind="ExternalInput")
    csx2 = nc.dram_tensor("csx2", (128, T), F16, kind="ExternalInput")
    mask4 = nc.dram_tensor("mask4", (128, 4 * TQ), F16, kind="ExternalInput")
    out = nc.dram_tensor("out", (T, E), F32, kind="ExternalOutput")

    with tile.TileContext(nc) as tc, nc.allow_low_precision(
        reason="fp16 matmul inputs / fp32r softmax stats are intentional"
    ):
        with tc.tile_pool(name="const", bufs=1) as const, \
             tc.tile_pool(name="qkt_p", bufs=1) as qkt_p, \
             tc.tile_pool(name="v_p", bufs=1) as v_p:
            cs_sb = const.tile([128, T], F16, tag="cs")    # [cos; sin]
            csd_sb = const.tile([128, T], F16, tag="csd")  # [sin; cos]
            mask_sb = const.tile([128, 4 * TQ], F16, tag="mask")

            # resident intermediates: QT/KT (transposed, de-interleaved, RoPE'd)
            # and V (natural layout)
            qkt_sb = [[qkt_p.tile([128, TQ], F16, tag=f"qkt{f}_{t}", name=f"qkt_sb{f}_{t}")
                       for t in range(NTQ)] for f in range(2 * HPC)]
            v_sb = [v_p.tile([128, FL], F16, tag=f"v{t}", name=f"v_sb{t}")
                    for t in range(NTK)]

            # ---------------- phase 1: QKV projection ----------------
            with tc.tile_pool(name="wqk_p", bufs=1) as wqk_p, \
                 tc.tile_pool(name="wv_p", bufs=1) as wv_p, \
                 tc.tile_pool(name="xt_p", bufs=3) as xt_p, \
                 tc.tile_pool(name="rope_t", bufs=3) as rope_t, \
                 tc.tile_pool(name="ps1", bufs=4, space="PSUM") as ps1:

                # DMA priority order: x-slabs + QK weights + RoPE tables gate
                # the first matmul chains; V weights / masks arrive later.
                wqk_sb = [wqk_p.tile([128, 2 * FL], F16, tag=f"wqk{e}", name=f"wqk_sb{e}")
                          for e in range(EC)]
                wv_sb = [wv_p.tile([128, FL], F16, tag=f"wv{e}", name=f"wv_sb{e}")
                         for e in range(EC)]

                for t4 in range(NTQ):
                    xt_sb = []
                    for e in range(EC):
                        xt = xt_p.tile([128, TQ], F16, tag=f"xt{e}")
                        nc.sync.dma_start(xt[:], xT[e * 128:(e + 1) * 128, t4 * TQ:(t4 + 1) * TQ])
                        xt_sb.append(xt)
                        if t4 == 0:
                            nc.sync.dma_start(wqk_sb[e][:], wqk[e])
                    if t4 == 0:
                        nc.sync.dma_start(cs_sb[:], csx[:])
                        nc.sync.dma_start(csd_sb[:], csx2[:])

                    # QT/KT blocks, transposed layout + RoPE on eviction
                    for f8 in range(2 * HPC):
                        ps = ps1.tile([128, TQ], F32, tag="qk")
                        for e in range(EC):
                            nc.tensor.matmul(
                                ps[:], wqk_sb[e][:, f8 * 128:(f8 + 1) * 128], xt_sb[e][:],
                                start=(e == 0), stop=(e == EC - 1),
                            )
                        ts4 = slice(t4 * TQ, (t4 + 1) * TQ)
                        dst = qkt_sb[f8][t4]
                        qk16 = rope_t.tile([128, TQ], F16, tag="qk16")
                        nc.scalar.copy(qk16[:], ps[:])
                        ve = nc.vector
                        t_a = rope_t.tile([64, TQ], F16, tag="ta")
                        t_b = rope_t.tile([64, TQ], F16, tag="tb")
                        ve.tensor_tensor(t_a[:], qk16[0:64, :], cs_sb[0:64, ts4], op=AluOpType.mult)
                        ve.tensor_tensor(t_b[:], qk16[64:128, :], cs_sb[64:128, ts4], op=AluOpType.mult)
                        ve.tensor_tensor(dst[0:64, :], t_a[:], t_b[:], op=AluOpType.subtract)
                        t_c = rope_t.tile([64, TQ], F16, tag="tc")
                        t_d = rope_t.tile([64, TQ], F16, tag="td")
                        ve.tensor_tensor(t_c[:], qk16[0:64, :], csd_sb[0:64, ts4], op=AluOpType.mult)
                        ve.tensor_tensor(t_d[:], qk16[64:128, :], csd_sb[64:128, ts4], op=AluOpType.mult)
                        ve.tensor_tensor(dst[64:128, :], t_c[:], t_d[:], op=AluOpType.add)

                    if t4 == 0:
                        for e in range(EC):
                            nc.sync.dma_start(wv_sb[e][:], wv[e])
                        nc.sync.dma_start(mask_sb[:], mask4[:])

                    # V row-tiles (natural layout [T, FL]) for this t4
                    for i in range(4):
                        tk = 4 * t4 + i
                        ps = ps1.tile([128, FL], F32, tag="v")
                        for e in range(EC):
                            nc.tensor.matmul(
                                ps[:], xt_sb[e][:, i * 128:(i + 1) * 128], wv_sb[e][:],
                                start=(e == 0), stop=(e == EC - 1),
                            )
                        nc.scalar.copy(v_sb[tk][:], ps[:])

            # ---------------- phase 2: per-head SDPA ----------------
            with tc.tile_pool(name="yt_p", bufs=1) as yt_p:
                yt_sb = [[yt_p.tile([128, TQ], F16, tag=f"yt{h}_{t}", name=f"yt_sb{h}_{t}")
                          for t in range(NTQ)] for h in range(HPC)]

                wo_p_cm = tc.tile_pool(name="wo_p", bufs=1)
                wo_p = wo_p_cm.__enter__()
                wo_sb = [wo_p.tile([128, E], F16, tag=f"wo{h}", name=f"wo_sb{h}")
                         for h in range(HPC)]
                for h in range(HPC):
                    nc.sync.dma_start(wo_sb[h][:], wout[h])

                with tc.tile_pool(name="es_p", bufs=14) as es_p, \
                     tc.tile_pool(name="dn_p", bufs=8) as dn_p, \
                     tc.tile_pool(name="pair_p", bufs=12) as pair_p, \
                     tc.tile_pool(name="o_ev", bufs=2) as o_ev, \
                     tc.tile_pool(name="ps_sc", bufs=2, space="PSUM") as ps_sc, \
                     tc.tile_pool(name="ps_y", bufs=2, space="PSUM") as ps_y, \
                     tc.tile_pool(name="ps3", bufs=2, space="PSUM") as ps3:

                    def sdpa_tq(tq, proj_after=()):
                        proj_after = list(proj_after)
                        for h in range(HPC):
                            nblk = 4 * tq + 4
                            qs = qkt_sb[h][tq][:]
                            yps = ps_y.tile([128, TQ], F32, tag="y", name=f"yps_{h}_{tq}")
                            pairs = []
                            for pk in range(nblk // 2):
                                spsP = ps_sc.tile([128, 2 * TQ], F32, tag="sc",
                                                  name=f"sps_{h}_{tq}_{pk}")
                                for half in (0, 1):
                                    tk = 2 * pk + half
                                    nc.tensor.matmul(
                                        spsP[:, half * TQ:(half + 1) * TQ],
                                        qkt_sb[HPC + h][tk // 4][:, (tk % 4) * 128:(tk % 4 + 1) * 128],
                                        qs, start=True, stop=True,
                                    )
                                esP = es_p.tile([128, 2 * TQ], F16, tag="es",
                                                name=f"es_{h}_{tq}_{pk}")
                                nc.scalar.activation(esP[:], spsP[:], EXP, scale=SCALE)
                                for half in (0, 1):
                                    tk = 2 * pk + half
                                    r = tk - 4 * tq
                                    if r >= 0:  # diagonal band: apply causal mask
                                        nc.vector.tensor_tensor(
                                            esP[:, half * TQ:(half + 1) * TQ],
                                            esP[:, half * TQ:(half + 1) * TQ],
                                            mask_sb[:, r * TQ:(r + 1) * TQ],
                                            op=AluOpType.mult,
                                        )
                                for half in (0, 1):
                                    tk = 2 * pk + half
                                    nc.tensor.matmul(
                                        yps[:], v_sb[tk][:, h * 128:(h + 1) * 128],
                                        esP[:, half * TQ:(half + 1) * TQ],
                                        start=(tk == 0), stop=(tk == nblk - 1),
                                        skip_group_check=True,
                                    )
                                pr = pair_p.tile([128, TQ], F16, tag="pr",
                                                 name=f"pr_{h}_{tq}_{pk}")
                                nc.vector.tensor_tensor(
                                    pr[:], esP[:, 0:TQ], esP[:, TQ:2 * TQ], op=AluOpType.add
                                )
                                pairs.append(pr)
                            dacc = dn_p.tile([128, TQ], F32, tag="dacc",
                                             name=f"dacc_{h}_{tq}")
                            nc.vector.tensor_tensor(dacc[:], pairs[0][:], pairs[1][:],
                                                    op=AluOpType.add)
                            for pr in pairs[2:]:
                                nc.vector.tensor_tensor(dacc[:], dacc[:], pr[:],
                                                        op=AluOpType.add)
                            denB = dn_p.tile([128, TQ], F32, tag="denB", name=f"denB_{h}_{tq}")
                            nc.gpsimd.partition_all_reduce(denB[:], dacc[:], channels=128,
                                                           reduce_op=bass_rust.ReduceOp.add)
                            rb = dn_p.tile([128, TQ], F32, tag="rb", name=f"rb_{h}_{tq}")
                            nc.vector.reciprocal_approx_fast(rb[:], denB[:])
                            nc.vector.tensor_tensor(
                                yt_sb[h][tq][:], yps[:], rb[:], op=AluOpType.mult,
                            )
                            if proj_after:
                                proj_range([proj_after.pop(0)])
                        proj_range(proj_after)

                    def proj_range(tqbs):
                        for tqb in tqbs:
                            osb = o_ev.tile([128, E], F32, tag="osb", name=f"osb_{tqb}")
                            for nb in range(NTQ):
                                ps = ps3.tile([128, TQ], F32, tag="o", name=f"ops_{tqb}_{nb}")
                                for h in range(HPC):
                                    nc.tensor.matmul(
                                        ps[:],
                                        yt_sb[h][tqb // 4][:, (tqb % 4) * 128:(tqb % 4 + 1) * 128],
                                        wo_sb[h][:, nb * TQ:(nb + 1) * TQ],
                                        start=(h == 0), stop=(h == HPC - 1),
                                    )
                                dsts = osb[:, nb * TQ:(nb + 1) * TQ]
                                if nb % 2 == 0:
                                    nc.scalar.copy(dsts, ps[:])
                                else:
                                    nc.vector.tensor_copy(dsts, ps[:])
                            nc.sync.dma_start(out[tqb * 128:(tqb + 1) * 128, :], osb[:])

                    # interleave projection chains between SDPA rounds so the
                    # PE stream has slack work while ACT/DVE drain the softmax
                    sdpa_tq(0)
                    sdpa_tq(1)
                    sdpa_tq(2, proj_after=range(0, 8))
                    sdpa_tq(3, proj_after=range(8, 16))
                wo_p_cm.__exit__(None, None, None)

    nc.compile()
    return nc


def _host_tables():
    positions = np.arange(T, dtype=np.float64)
    inv_freq = 1.0 / (THETA ** (np.arange(0, D, 2, dtype=np.float64) / D))
    freqs = np.outer(positions, inv_freq)          # [T, 64]
    cs = np.concatenate([np.cos(freqs).T, np.sin(freqs).T]).astype(np.float16)   # [128, T]
    cs2 = np.concatenate([np.sin(freqs).T, np.cos(freqs).T]).astype(np.float16)  # swapped halves
    masks = np.zeros((4, 128, TQ), dtype=np.float16)
    p = np.arange(128)[:, None]
    f = np.arange(TQ)[None, :]
    for r in range(4):
        masks[r] = (128 * r + p <= f).astype(np.float16)
    masks2d = np.ascontiguousarray(masks.transpose(1, 0, 2).reshape(128, 4 * TQ))
    return cs, cs2, masks2d


def kernel(x, W_qkv, W_out):
    global _compiled
    if _compiled is None:
        _compiled = _build()
    nc = _compiled

    x = np.ascontiguousarray(np.asarray(x, dtype=np.float32))
    W_qkv = np.asarray(W_qkv, dtype=np.float32)
    W_out = np.asarray(W_out, dtype=np.float32)

    cs, cs2, masks = _host_tables()

    perm = np.concatenate([np.arange(0, D, 2), np.arange(1, D, 2)])  # de-interleave

    in_maps = []
    for c in range(N_CORES):
        b, tp = divmod(c, TP)
        heads = np.arange(tp * HPC, (tp + 1) * HPC)
        qk_cols = np.concatenate(
            [h * D + perm for h in heads] + [E + h * D + perm for h in heads]
        )
        v_cols = np.concatenate([2 * E + h * D + np.arange(D) for h in heads])
        wqk_l = np.ascontiguousarray(W_qkv[:, qk_cols]).reshape(EC, 128, 2 * FL)
        wv_l = np.ascontiguousarray(W_qkv[:, v_cols]).reshape(EC, 128, FL)
        wout_l = np.ascontiguousarray(
            W_out.reshape(N_HEAD, D, E)[heads].reshape(HPC, 128, E)
        )
        in_maps.append({
            "xT": np.ascontiguousarray(x[b].T).astype(np.float16),
            "wqk": wqk_l.astype(np.float16),
            "wv": wv_l.astype(np.float16),
            "wout": wout_l.astype(np.float16),
            "csx": cs,
            "csx2": cs2,
            "mask4": masks,
        })

    global _last_in_maps
    _last_in_maps = in_maps
    res = bass_utils.run_bass_kernel_spmd(nc, in_maps, core_ids=list(range(N_CORES)))
    out = np.zeros((B, T, E), dtype=np.float32)
    for c in range(N_CORES):
        out[c // TP] += res.results[c]["out"]
    return out


# revision 37
# speedup vs baseline: 1.1052x; 1.1052x over previous
"""Trainium2 Bass kernel for a causal attention block (B=2, T=2048, E=2048,
16 heads, head_dim=128, interleaved RoPE).

Sharding: data-parallel over batch (2) x tensor-parallel over heads (4 per
core) = 8 NeuronCores. Each core computes QKV projection for its 4 heads,
RoPE, causal SDPA, and a partial output projection (row-sharded W_out); the
host sums the 4 TP partials per batch element.

Per-core dataflow (fp16 matmul inputs, fp32 PSUM accumulation):
  phase 1: QT/KT computed transposed ([head_dim, T], rows de-interleaved so
           RoPE is a top/bottom-half rotation applied at PSUM eviction) and
           V natural [T, head_dim]; all stay resident in SBUF.
  phase 2: per head: scoresT[Tk,Tq] = KT_tile.T @ QT, exp (scale folded in),
           causal mask on the diagonal blocks, denominator accumulated on
           DVE + ones-matmul + fast reciprocal + gpsimd partition broadcast;
           exp-scores feed Y.T[D,Tq] = V.T @ A.T directly, normalized on
           eviction.
  phase 3: out[T, E] partial = Y.T (as lhsT) against W_out local rows.
"""

import sys

sys.path.insert(0, "/opt/trn_rl_repo")

import numpy as np

import bass_rust
import concourse.bacc as bacc
import concourse.mybir as mybir
from concourse.alu_op_type import AluOpType
from concourse import tile
from concourse import bass_utils

B, T, E = 2, 2048, 2048
N_HEAD = 16
D = E // N_HEAD            # 128
THETA = 10000.0
N_CORES = 8
TP = 4                     # tensor-parallel degree (heads)
HPC = N_HEAD // TP         # heads per core = 4
FL = HPC * D               # local head width = 512
EC = E // 128              # 16 contraction chunks
TQ = 512                   # query tile (free dim)
NTQ = T // TQ              # 4
NTK = T // 128             # 16

F32 = mybir.dt.float32
F32R = mybir.dt.float32r
F16 = mybir.dt.float16
EXP = mybir.ActivationFunctionType.Exp
SCALE = 1.0 / np.sqrt(D)

_compiled = None
_last_in_maps = None


def _build():
    nc = bacc.Bacc("TRN2", target_bir_lowering=False)

    xT = nc.dram_tensor("xT", (E, T), F16, kind="ExternalInput")
    wqk = nc.dram_tensor("wqk", (EC, 128, 2 * FL), F16, kind="ExternalInput")
    wv = nc.dram_tensor("wv", (EC, 128, FL), F16, kind="ExternalInput")
    wout = nc.dram_tensor("wout", (HPC, 128, E), F16, kind="ExternalInput")
    csx = nc.dram_tensor("csx", (128, T), F16, kind="ExternalInput")
    csx2 = nc.dram_tensor("csx2", (128, T), F16, kind="ExternalInput")
    mask4 = nc.dram_tensor("mask4", (128, 4 * TQ), F16, kind="ExternalInput")
    ones_col = nc.dram_tensor("ones_col", (128, 1), F32R, kind="ExternalInput")
    out = nc.dram_tensor("out", (T, E), F32, kind="ExternalOutput")

    with tile.TileContext(nc) as tc, nc.allow_low_precision(
        reason="fp16 matmul inputs / fp32r softmax stats are intentional"
    ):
        with tc.tile_pool(name="const", bufs=1) as const, \
             tc.tile_pool(name="qkt_p", bufs=1) as qkt_p, \
             tc.tile_pool(name="v_p", bufs=1) as v_p:
            cs_sb = const.tile([128, T], F16, tag="cs")    # [cos; sin]
            csd_sb = const.tile([128, T], F16, tag="csd")  # [sin; cos]
            mask_sb = const.tile([128, 4 * TQ], F16, tag="mask")
            onec = const.tile([128, 1], F32R, tag="onec")

            # resident intermediates: QT/KT (transposed, de-interleaved, RoPE'd)
            # and V (natural layout)
            qkt_sb = [[qkt_p.tile([128, TQ], F16, tag=f"qkt{f}_{t}", name=f"qkt_sb{f}_{t}")
                       for t in range(NTQ)] for f in range(2 * HPC)]
            v_sb = [v_p.tile([128, FL], F16, tag=f"v{t}", name=f"v_sb{t}")
                    for t in range(NTK)]

            # ---------------- phase 1: QKV projection ----------------
            with tc.tile_pool(name="wqk_p", bufs=1) as wqk_p, \
                 tc.tile_pool(name="wv_p", bufs=1) as wv_p, \
                 tc.tile_pool(name="xt_p", bufs=3) as xt_p, \
                 tc.tile_pool(name="rope_t", bufs=3) as rope_t, \
                 tc.tile_pool(name="ps1", bufs=4, space="PSUM") as ps1:

                # DMA priority order: x-slabs + QK weights + RoPE tables gate
                # the first matmul chains; V weights / masks arrive later.
                wqk_sb = [wqk_p.tile([128, 2 * FL], F16, tag=f"wqk{e}", name=f"wqk_sb{e}")
                          for e in range(EC)]
                wv_sb = [wv_p.tile([128, FL], F16, tag=f"wv{e}", name=f"wv_sb{e}")
                         for e in range(EC)]

                for t4 in range(NTQ):
                    xt_sb = []
                    for e in range(EC):
                        xt = xt_p.tile([128, TQ], F16, tag=f"xt{e}")
                        nc.sync.dma_start(xt[:], xT[e * 128:(e + 1) * 128, t4 * TQ:(t4 + 1) * TQ])
                        xt_sb.append(xt)
                        if t4 == 0:
                            nc.sync.dma_start(wqk_sb[e][:], wqk[e])
                    if t4 == 0:
                        nc.sync.dma_start(cs_sb[:], csx[:])
                        nc.sync.dma_start(csd_sb[:], csx2[:])

                    # QT/KT blocks, transposed layout + RoPE on eviction
                    for f8 in range(2 * HPC):
                        ps = ps1.tile([128, TQ], F32, tag="qk")
                        for e in range(EC):
                            nc.tensor.matmul(
                                ps[:], wqk_sb[e][:, f8 * 128:(f8 + 1) * 128], xt_sb[e][:],
                                start=(e == 0), stop=(e == EC - 1),
                            )
                        ts4 = slice(t4 * TQ, (t4 + 1) * TQ)
                        dst = qkt_sb[f8][t4]
                        qk16 = rope_t.tile([128, TQ], F16, tag="qk16")
                        nc.scalar.copy(qk16[:], ps[:])
                        t_a = rope_t.tile([64, TQ], F16, tag="ta")
                        t_b = rope_t.tile([64, TQ], F16, tag="tb")
                        nc.vector.tensor_tensor(t_a[:], qk16[0:64, :], cs_sb[0:64, ts4], op=AluOpType.mult)
                        nc.vector.tensor_tensor(t_b[:], qk16[64:128, :], cs_sb[64:128, ts4], op=AluOpType.mult)
                        nc.vector.tensor_tensor(dst[0:64, :], t_a[:], t_b[:], op=AluOpType.subtract)
                        t_c = rope_t.tile([64, TQ], F16, tag="tc")
                        t_d = rope_t.tile([64, TQ], F16, tag="td")
                        nc.vector.tensor_tensor(t_c[:], qk16[0:64, :], csd_sb[0:64, ts4], op=AluOpType.mult)
                        nc.vector.tensor_tensor(t_d[:], qk16[64:128, :], csd_sb[64:128, ts4], op=AluOpType.mult)
                        nc.vector.tensor_tensor(dst[64:128, :], t_c[:], t_d[:], op=AluOpType.add)

                    if t4 == 0:
                        for e in range(EC):
                            nc.sync.dma_start(wv_sb[e][:], wv[e])
                        nc.sync.dma_start(mask_sb[:], mask4[:])
                        nc.sync.dma_start(onec[:], ones_col[:])

                    # V row-tiles (natural layout [T, FL]) for this t4
                    for i in range(4):
                        tk = 4 * t4 + i
                        ps = ps1.tile([128, FL], F32, tag="v")
                        for e in range(EC):
                            nc.tensor.matmul(
                                ps[:], xt_sb[e][:, i * 128:(i + 1) * 128], wv_sb[e][:],
                                start=(e == 0), stop=(e == EC - 1),
                            )
                        nc.scalar.copy(v_sb[tk][:], ps[:])

            # ---------------- phase 2: per-head SDPA ----------------
            with tc.tile_pool(name="yt_p", bufs=1) as yt_p:
                yt_sb = [[yt_p.tile([128, TQ], F16, tag=f"yt{h}_{t}", name=f"yt_sb{h}_{t}")
                          for t in range(NTQ)] for h in range(HPC)]

                wo_p_cm = tc.tile_pool(name="wo_p", bufs=1)
                wo_p = wo_p_cm.__enter__()
                wo_sb = [wo_p.tile([128, E], F16, tag=f"wo{h}", name=f"wo_sb{h}")
                         for h in range(HPC)]
                for h in range(HPC):
                    nc.sync.dma_start(wo_sb[h][:], wout[h])

                with tc.tile_pool(name="es_p", bufs=12) as es_p, \
                     tc.tile_pool(name="dn_p", bufs=5) as dn_p, \
                     tc.tile_pool(name="pair_p", bufs=10) as pair_p, \
                     tc.tile_pool(name="o_ev", bufs=2) as o_ev, \
                     tc.tile_pool(name="ps_sc", bufs=3, space="PSUM") as ps_sc, \
                     tc.tile_pool(name="ps_y", bufs=2, space="PSUM") as ps_y, \
                     tc.tile_pool(name="ps_sm", bufs=1, space="PSUM") as ps_sm, \
                     tc.tile_pool(name="ps3", bufs=2, space="PSUM") as ps3:

                    def sdpa_tq(tq, proj_after=()):
                        proj_after = list(proj_after)
                        for h in range(HPC):
                            nblk = 4 * tq + 4
                            qs = qkt_sb[h][tq][:]
                            yps = ps_y.tile([128, TQ], F32, tag="y", name=f"yps_{h}_{tq}")
                            pairs = []
                            for tk in range(nblk):
                                sps = ps_sc.tile([128, TQ], F32, tag="sc",
                                                 name=f"sps_{h}_{tq}_{tk}")
                                nc.tensor.matmul(
                                    sps[:],
                                    qkt_sb[HPC + h][tk // 4][:, (tk % 4) * 128:(tk % 4 + 1) * 128],
                                    qs, start=True, stop=True,
                                )
                                es = es_p.tile([128, TQ], F16, tag="es",
                                               name=f"es_{h}_{tq}_{tk}")
                                nc.scalar.activation(es[:], sps[:], EXP, scale=SCALE)
                                r = tk - 4 * tq
                                if r >= 0:  # diagonal band: apply causal mask
                                    nc.vector.tensor_tensor(
                                        es[:], es[:], mask_sb[:, r * TQ:(r + 1) * TQ],
                                        op=AluOpType.mult,
                                    )
                                nc.tensor.matmul(
                                    yps[:], v_sb[tk][:, h * 128:(h + 1) * 128], es[:],
                                    start=(tk == 0), stop=(tk == nblk - 1),
                                    skip_group_check=True,
                                )
                                # denominator: fp16 pair sums (2x DVE mode), then
                                # a short fp32r combine chain
                                if tk % 2 == 1:
                                    pr = pair_p.tile([128, TQ], F16, tag="pr",
                                                     name=f"pr_{h}_{tq}_{tk}")
                                    nc.vector.tensor_tensor(
                                        pr[:], es_prev[:], es[:], op=AluOpType.add
                                    )
                                    pairs.append(pr)
                                es_prev = es
                            dacc = dn_p.tile([128, TQ], F32R, tag="dacc",
                                             name=f"dacc_{h}_{tq}")
                            nc.vector.tensor_tensor(dacc[:], pairs[0][:], pairs[1][:],
                                                    op=AluOpType.add)
                            for pr in pairs[2:]:
                                nc.vector.tensor_tensor(dacc[:], dacc[:], pr[:],
                                                        op=AluOpType.add)
                            dps = ps_sm.tile([1, TQ], F32, tag="dr", name=f"dps_{h}_{tq}")
                            nc.tensor.matmul(dps[:], onec[:], dacc[:], start=True, stop=True)
                            rrow = dn_p.tile([1, TQ], F32, tag="rrow", name=f"rrow_{h}_{tq}")
                            nc.vector.reciprocal_approx_fast(rrow[:], dps[:])
                            rb = dn_p.tile([128, TQ], F32, tag="rb", name=f"rb_{h}_{tq}")
                            nc.gpsimd.partition_broadcast(rb[:], rrow[:])
                            nc.vector.tensor_tensor(
                                yt_sb[h][tq][:], yps[:], rb[:], op=AluOpType.mult,
                            )
                            if proj_after:
                                proj_range([proj_after.pop(0)])
                        proj_range(proj_after)

                    def proj_range(tqbs):
                        for tqb in tqbs:
                            osb = o_ev.tile([128, E], F32, tag="osb", name=f"osb_{tqb}")
                            for nb in range(NTQ):
                                ps = ps3.tile([128, TQ], F32, tag="o", name=f"ops_{tqb}_{nb}")
                                for h in range(HPC):
                                    nc.tensor.matmul(
                                        ps[:],
                                        yt_sb[h][tqb // 4][:, (tqb % 4) * 128:(tqb % 4 + 1) * 128],
                                        wo_sb[h][:, nb * TQ:(nb + 1) * TQ],
                                        start=(h == 0), stop=(h == HPC - 1),
                                    )
                                dsts = osb[:, nb * TQ:(nb + 1) * TQ]
                                if nb % 2 == 0:
                                    nc.scalar.copy(dsts, ps[:])
                                else:
                                    nc.vector.tensor_copy(dsts, ps[:])
                            nc.sync.dma_start(out[tqb * 128:(tqb + 1) * 128, :], osb[:])

                    # interleave projection chains between SDPA rounds so the
                    # PE stream has slack work while ACT/DVE drain the softmax
                    sdpa_tq(0)
                    sdpa_tq(1)
                    sdpa_tq(2, proj_after=range(0, 8))
                    sdpa_tq(3, proj_after=range(8, 16))
                wo_p_cm.__exit__(None, None, None)

    nc.compile()
    return nc


def _host_tables():
    positions = np.arange(T, dtype=np.float64)
    inv_freq = 1.0 / (THETA ** (np.arange(0, D, 2, dtype=np.float64) / D))
    freqs = np.outer(positions, inv_freq)          # [T, 64]
    cs = np.concatenate([np.cos(freqs).T, np.sin(freqs).T]).astype(np.float16)   # [128, T]
    cs2 = np.concatenate([np.sin(freqs).T, np.cos(freqs).T]).astype(np.float16)  # swapped halves
    masks = np.zeros((4, 128, TQ), dtype=np.float16)
    p = np.arange(128)[:, None]
    f = np.arange(TQ)[None, :]
    for r in range(4):
        masks[r] = (128 * r + p <= f).astype(np.float16)
    masks2d = np.ascontiguousarray(masks.transpose(1, 0, 2).reshape(128, 4 * TQ))
    return cs, cs2, masks2d


def kernel(x, W_qkv, W_out):
    global _compiled
    if _compiled is None:
        _compiled = _build()
    nc = _compiled

    x = np.ascontiguousarray(np.asarray(x, dtype=np.float32))
    W_qkv = np.asarray(W_qkv, dtype=np.float32)
    W_out = np.asarray(W_out, dtype=np.float32)

    cs, cs2, masks = _host_tables()
    ones_c = np.ones((128, 1), np.float32)

    perm = np.concatenate([np.arange(0, D, 2), np.arange(1, D, 2)])  # de-interleave

    in_maps = []
    for c in range(N_CORES):
        b, tp = divmod(c, TP)
        heads = np.arange(tp * HPC, (tp + 1) * HPC)
        qk_cols = np.concatenate(
            [h * D + perm for h in heads] + [E + h * D + perm for h in heads]
        )
        v_cols = np.concatenate([2 * E + h * D + np.arange(D) for h in heads])
        wqk_l = np.ascontiguousarray(W_qkv[:, qk_cols]).reshape(EC, 128, 2 * FL)
        wv_l = np.ascontiguousarray(W_qkv[:, v_cols]).reshape(EC, 128, FL)
        wout_l = np.ascontiguousarray(
            W_out.reshape(N_HEAD, D, E)[heads].reshape(HPC, 128, E)
        )
        in_maps.append({
            "xT": np.ascontiguousarray(x[b].T).astype(np.float16),
            "wqk": wqk_l.astype(np.float16),
            "wv": wv_l.astype(np.float16),
            "wout": wout_l.astype(np.float16),
            "csx": cs,
            "csx2": cs2,
            "mask4": masks,
            "ones_col": ones_c,
        })

    global _last_in_maps
    _last_in_maps = in_maps
    res = bass_utils.run_bass_kernel_spmd(nc, in_maps, core_ids=list(range(N_CORES)))
    out = np.zeros((B, T, E), dtype=np.float32)
    for c in range(N_CORES):
        out[c // TP] += res.results[c]["out"]
    return out


# revision 38
# speedup vs baseline: 1.1080x; 1.0025x over previous
"""Trainium2 Bass kernel for a causal attention block (B=2, T=2048, E=2048,
16 heads, head_dim=128, interleaved RoPE).

Sharding: data-parallel over batch (2) x tensor-parallel over heads (4 per
core) = 8 NeuronCores. Each core computes QKV projection for its 4 heads,
RoPE, causal SDPA, and a partial output projection (row-sharded W_out); the
host sums the 4 TP partials per batch element.

Per-core dataflow (fp16 matmul inputs, fp32 PSUM accumulation):
  phase 1: QT/KT computed transposed ([head_dim, T], rows de-interleaved so
           RoPE is a top/bottom-half rotation applied at PSUM eviction) and
           V natural [T, head_dim]; all stay resident in SBUF.
  phase 2: per head: scoresT[Tk,Tq] = KT_tile.T @ QT, exp (scale folded in),
           causal mask on the diagonal blocks, denominator accumulated on
           DVE + ones-matmul + fast reciprocal + gpsimd partition broadcast;
           exp-scores feed Y.T[D,Tq] = V.T @ A.T directly, normalized on
           eviction.
  phase 3: out[T, E] partial = Y.T (as lhsT) against W_out local rows.
"""

import sys

sys.path.insert(0, "/opt/trn_rl_repo")

import numpy as np

import bass_rust
import concourse.bacc as bacc
import concourse.mybir as mybir
from concourse.alu_op_type import AluOpType
from concourse import tile
from concourse import bass_utils

B, T, E = 2, 2048, 2048
N_HEAD = 16
D = E // N_HEAD            # 128
THETA = 10000.0
N_CORES = 8
TP = 4                     # tensor-parallel degree (heads)
HPC = N_HEAD // TP         # heads per core = 4
FL = HPC * D               # local head width = 512
EC = E // 128              # 16 contraction chunks
TQ = 512                   # query tile (free dim)
NTQ = T // TQ              # 4
NTK = T // 128             # 16

F32 = mybir.dt.float32
F32R = mybir.dt.float32r
F16 = mybir.dt.float16
EXP = mybir.ActivationFunctionType.Exp
SCALE = 1.0 / np.sqrt(D)

_compiled = None
_last_in_maps = None


def _build():
    nc = bacc.Bacc("TRN2", target_bir_lowering=False)

    xT = nc.dram_tensor("xT", (E, T), F16, kind="ExternalInput")
    wqk = nc.dram_tensor("wqk", (EC, 128, 2 * FL), F16, kind="ExternalInput")
    wv = nc.dram_tensor("wv", (EC, 128, FL), F16, kind="ExternalInput")
    wout = nc.dram_tensor("wout", (HPC, 128, E), F16, kind="ExternalInput")
    csx = nc.dram_tensor("csx", (128, T), F16, kind="ExternalInput")
    csx2 = nc.dram_tensor("csx2", (128, T), F16, kind="ExternalInput")
    mask4 = nc.dram_tensor("mask4", (128, 4 * TQ), F16, kind="ExternalInput")
    ones_col = nc.dram_tensor("ones_col", (128, 1), F32R, kind="ExternalInput")
    out = nc.dram_tensor("out", (T, E), F32, kind="ExternalOutput")

    with tile.TileContext(nc) as tc, nc.allow_low_precision(
        reason="fp16 matmul inputs / fp32r softmax stats are intentional"
    ):
        with tc.tile_pool(name="const", bufs=1) as const, \
             tc.tile_pool(name="qkt_p", bufs=1) as qkt_p, \
             tc.tile_pool(name="v_p", bufs=1) as v_p:
            cs_sb = const.tile([128, T], F16, tag="cs")    # [cos; sin]
            csd_sb = const.tile([128, T], F16, tag="csd")  # [sin; cos]
            mask_sb = const.tile([128, 4 * TQ], F16, tag="mask")
            onec = const.tile([128, 1], F32R, tag="onec")

            # resident intermediates: QT/KT (transposed, de-interleaved, RoPE'd)
            # and V (natural layout)
            qkt_sb = [[qkt_p.tile([128, TQ], F16, tag=f"qkt{f}_{t}", name=f"qkt_sb{f}_{t}")
                       for t in range(NTQ)] for f in range(2 * HPC)]
            v_sb = [v_p.tile([128, FL], F16, tag=f"v{t}", name=f"v_sb{t}")
                    for t in range(NTK)]

            # ---------------- phase 1: QKV projection ----------------
            with tc.tile_pool(name="wqk_p", bufs=1) as wqk_p, \
                 tc.tile_pool(name="wv_p", bufs=1) as wv_p, \
                 tc.tile_pool(name="xt_p", bufs=3) as xt_p, \
                 tc.tile_pool(name="rope_t", bufs=6) as rope_t, \
                 tc.tile_pool(name="ps1", bufs=4, space="PSUM") as ps1:

                # DMA priority order: x-slabs + QK weights + RoPE tables gate
                # the first matmul chains; V weights / masks arrive later.
                wqk_sb = [wqk_p.tile([128, 2 * FL], F16, tag=f"wqk{e}", name=f"wqk_sb{e}")
                          for e in range(EC)]
                wv_sb = [wv_p.tile([128, FL], F16, tag=f"wv{e}", name=f"wv_sb{e}")
                         for e in range(EC)]

                for t4 in range(NTQ):
                    xt_sb = []
                    for e in range(EC):
                        xt = xt_p.tile([128, TQ], F16, tag=f"xt{e}")
                        nc.sync.dma_start(xt[:], xT[e * 128:(e + 1) * 128, t4 * TQ:(t4 + 1) * TQ])
                        xt_sb.append(xt)
                        if t4 == 0:
                            nc.sync.dma_start(wqk_sb[e][:], wqk[e])
                    if t4 == 0:
                        nc.sync.dma_start(cs_sb[:], csx[:])
                        nc.sync.dma_start(csd_sb[:], csx2[:])

                    # QT/KT blocks, transposed layout + RoPE on eviction
                    for f8 in range(2 * HPC):
                        ps = ps1.tile([128, TQ], F32, tag="qk")
                        for e in range(EC):
                            nc.tensor.matmul(
                                ps[:], wqk_sb[e][:, f8 * 128:(f8 + 1) * 128], xt_sb[e][:],
                                start=(e == 0), stop=(e == EC - 1),
                            )
                        ts4 = slice(t4 * TQ, (t4 + 1) * TQ)
                        dst = qkt_sb[f8][t4]
                        qk16 = rope_t.tile([128, TQ], F16, tag="qk16")
                        nc.scalar.copy(qk16[:], ps[:])
                        t_a = rope_t.tile([64, TQ], F16, tag="ta")
                        t_b = rope_t.tile([64, TQ], F16, tag="tb")
                        nc.vector.tensor_tensor(t_a[:], qk16[0:64, :], cs_sb[0:64, ts4], op=AluOpType.mult)
                        nc.vector.tensor_tensor(t_b[:], qk16[64:128, :], cs_sb[64:128, ts4], op=AluOpType.mult)
                        nc.vector.tensor_tensor(dst[0:64, :], t_a[:], t_b[:], op=AluOpType.subtract)
                        t_c = rope_t.tile([64, TQ], F16, tag="tc")
                        t_d = rope_t.tile([64, TQ], F16, tag="td")
                        nc.vector.tensor_tensor(t_c[:], qk16[0:64, :], csd_sb[0:64, ts4], op=AluOpType.mult)
                        nc.vector.tensor_tensor(t_d[:], qk16[64:128, :], csd_sb[64:128, ts4], op=AluOpType.mult)
                        nc.vector.tensor_tensor(dst[64:128, :], t_c[:], t_d[:], op=AluOpType.add)

                    if t4 == 0:
                        for e in range(EC):
                            nc.sync.dma_start(wv_sb[e][:], wv[e])
                        nc.sync.dma_start(mask_sb[:], mask4[:])
                        nc.sync.dma_start(onec[:], ones_col[:])

                    # V row-tiles (natural layout [T, FL]) for this t4
                    for i in range(4):
                        tk = 4 * t4 + i
                        ps = ps1.tile([128, FL], F32, tag="v")
                        for e in range(EC):
                            nc.tensor.matmul(
                                ps[:], xt_sb[e][:, i * 128:(i + 1) * 128], wv_sb[e][:],
                                start=(e == 0), stop=(e == EC - 1),
                            )
                        nc.scalar.copy(v_sb[tk][:], ps[:])

            # ---------------- phase 2: per-head SDPA ----------------
            with tc.tile_pool(name="yt_p", bufs=1) as yt_p:
                yt_sb = [[yt_p.tile([128, TQ], F16, tag=f"yt{h}_{t}", name=f"yt_sb{h}_{t}")
                          for t in range(NTQ)] for h in range(HPC)]

                wo_p_cm = tc.tile_pool(name="wo_p", bufs=1)
                wo_p = wo_p_cm.__enter__()
                wo_sb = [wo_p.tile([128, E], F16, tag=f"wo{h}", name=f"wo_sb{h}")
                         for h in range(HPC)]
                for h in range(HPC):
                    nc.sync.dma_start(wo_sb[h][:], wout[h])

                with tc.tile_pool(name="es_p", bufs=12) as es_p, \
                     tc.tile_pool(name="dn_p", bufs=5) as dn_p, \
                     tc.tile_pool(name="pair_p", bufs=10) as pair_p, \
                     tc.tile_pool(name="o_ev", bufs=2) as o_ev, \
                     tc.tile_pool(name="ps_sc", bufs=3, space="PSUM") as ps_sc, \
                     tc.tile_pool(name="ps_y", bufs=2, space="PSUM") as ps_y, \
                     tc.tile_pool(name="ps_sm", bufs=1, space="PSUM") as ps_sm, \
                     tc.tile_pool(name="ps3", bufs=2, space="PSUM") as ps3:

                    def sdpa_tq(tq, proj_after=()):
                        proj_after = list(proj_after)
                        for h in range(HPC):
                            nblk = 4 * tq + 4
                            qs = qkt_sb[h][tq][:]
                            yps = ps_y.tile([128, TQ], F32, tag="y", name=f"yps_{h}_{tq}")
                            pairs = []
                            for tk in range(nblk):
                                sps = ps_sc.tile([128, TQ], F32, tag="sc",
                                                 name=f"sps_{h}_{tq}_{tk}")
                                nc.tensor.matmul(
                                    sps[:],
                                    qkt_sb[HPC + h][tk // 4][:, (tk % 4) * 128:(tk % 4 + 1) * 128],
                                    qs, start=True, stop=True,
                                )
                                es = es_p.tile([128, TQ], F16, tag="es",
                                               name=f"es_{h}_{tq}_{tk}")
                                nc.scalar.activation(es[:], sps[:], EXP, scale=SCALE)
                                r = tk - 4 * tq
                                if r >= 0:  # diagonal band: apply causal mask
                                    nc.vector.tensor_tensor(
                                        es[:], es[:], mask_sb[:, r * TQ:(r + 1) * TQ],
                                        op=AluOpType.mult,
                                    )
                                nc.tensor.matmul(
                                    yps[:], v_sb[tk][:, h * 128:(h + 1) * 128], es[:],
                                    start=(tk == 0), stop=(tk == nblk - 1),
                                    skip_group_check=True,
                                )
                                # denominator: fp16 pair sums (2x DVE mode), then
                                # a short fp32r combine chain
                                if tk % 2 == 1:
                                    pr = pair_p.tile([128, TQ], F16, tag="pr",
                                                     name=f"pr_{h}_{tq}_{tk}")
                                    nc.vector.tensor_tensor(
                                        pr[:], es_prev[:], es[:], op=AluOpType.add
                                    )
                                    pairs.append(pr)
                                es_prev = es
                            dacc = dn_p.tile([128, TQ], F32R, tag="dacc",
                                             name=f"dacc_{h}_{tq}")
                            nc.vector.tensor_tensor(dacc[:], pairs[0][:], pairs[1][:],
                                                    op=AluOpType.add)
                            for pr in pairs[2:]:
                                nc.vector.tensor_tensor(dacc[:], dacc[:], pr[:],
                                                        op=AluOpType.add)
                            dps = ps_sm.tile([1, TQ], F32, tag="dr", name=f"dps_{h}_{tq}")
                            nc.tensor.matmul(dps[:], onec[:], dacc[:], start=True, stop=True)
                            rrow = dn_p.tile([1, TQ], F32, tag="rrow", name=f"rrow_{h}_{tq}")
                            nc.vector.reciprocal_approx_fast(rrow[:], dps[:])
                            rb = dn_p.tile([128, TQ], F32, tag="rb", name=f"rb_{h}_{tq}")
                            nc.gpsimd.partition_broadcast(rb[:], rrow[:])
                            nc.vector.tensor_tensor(
                                yt_sb[h][tq][:], yps[:], rb[:], op=AluOpType.mult,
                            )
                            if proj_after:
                                proj_range([proj_after.pop(0)])
                        proj_range(proj_after)

                    def proj_range(tqbs):
                        for tqb in tqbs:
                            osb = o_ev.tile([128, E], F32, tag="osb", name=f"osb_{tqb}")
                            for nb in range(NTQ):
                                ps = ps3.tile([128, TQ], F32, tag="o", name=f"ops_{tqb}_{nb}")
                                for h in range(HPC):
                                    nc.tensor.matmul(
                                        ps[:],
                                        yt_sb[h][tqb // 4][:, (tqb % 4) * 128:(tqb % 4 + 1) * 128],
                                        wo_sb[h][:, nb * TQ:(nb + 1) * TQ],
                                        start=(h == 0), stop=(h == HPC - 1),
                                    )
                                dsts = osb[:, nb * TQ:(nb + 1) * TQ]
                                if nb % 2 == 0:
                                    nc.scalar.copy(dsts, ps[:])
                                else:
                                    nc.vector.tensor_copy(dsts, ps[:])
                            nc.sync.dma_start(out[tqb * 128:(tqb + 1) * 128, :], osb[:])

                    # interleave projection chains between SDPA rounds so the
                    # PE stream has slack work while ACT/DVE drain the softmax
                    sdpa_tq(0)
                    sdpa_tq(1)
                    sdpa_tq(2, proj_after=range(0, 8))
                    sdpa_tq(3, proj_after=range(8, 16))
                wo_p_cm.__exit__(None, None, None)

    nc.compile()
    return nc


def _host_tables():
    positions = np.arange(T, dtype=np.float64)
    inv_freq = 1.0 / (THETA ** (np.arange(0, D, 2, dtype=np.float64) / D))
    freqs = np.outer(positions, inv_freq)          # [T, 64]
    cs = np.concatenate([np.cos(freqs).T, np.sin(freqs).T]).astype(np.float16)   # [128, T]
    cs2 = np.concatenate([np.sin(freqs).T, np.cos(freqs).T]).astype(np.float16)  # swapped halves
    masks = np.zeros((4, 128, TQ), dtype=np.float16)
    p = np.arange(128)[:, None]
    f = np.arange(TQ)[None, :]
    for r in range(4):
        masks[r] = (128 * r + p <= f).astype(np.float16)
    masks2d = np.ascontiguousarray(masks.transpose(1, 0, 2).reshape(128, 4 * TQ))
    return cs, cs2, masks2d


def kernel(x, W_qkv, W_out):
    global _compiled
    if _compiled is None:
        _compiled = _build()
    nc = _compiled

    x = np.ascontiguousarray(np.asarray(x, dtype=np.float32))
    W_qkv = np.asarray(W_qkv, dtype=np.float32)
    W_out = np.asarray(W_out, dtype=np.float32)

    cs, cs2, masks = _host_tables()
    ones_c = np.ones((128, 1), np.float32)

    perm = np.concatenate([np.arange(0, D, 2), np.arange(1, D, 2)])  # de-interleave

    in_maps = []
    for c in range(N_CORES):
        b, tp = divmod(c, TP)
        heads = np.arange(tp * HPC, (tp + 1) * HPC)
        qk_cols = np.concatenate(
            [h * D + perm for h in heads] + [E + h * D + perm for h in heads]
        )
        v_cols = np.concatenate([2 * E + h * D + np.arange(D) for h in heads])
        wqk_l = np.ascontiguousarray(W_qkv[:, qk_cols]).reshape(EC, 128, 2 * FL)
        wv_l = np.ascontiguousarray(W_qkv[:, v_cols]).reshape(EC, 128, FL)
        wout_l = np.ascontiguousarray(
            W_out.reshape(N_HEAD, D, E)[heads].reshape(HPC, 128, E)
        )
        in_maps.append({
            "xT": np.ascontiguousarray(x[b].T).astype(np.float16),
            "wqk": wqk_l.astype(np.float16),
            "wv": wv_l.astype(np.float16),
            "wout": wout_l.astype(np.float16),
            "csx": cs,
            "csx2": cs2,
            "mask4": masks,
            "ones_col": ones_c,
        })

    global _last_in_maps
    _last_in_maps = in_maps
    res = bass_utils.run_bass_kernel_spmd(nc, in_maps, core_ids=list(range(N_CORES)))
    out = np.zeros((B, T, E), dtype=np.float32)
    for c in range(N_CORES):
        out[c // TP] += res.results[c]["out"]
    return out
